# revision 3
# baseline (speedup 1.0000x reference)
"""LIF-with-residue Trainium2 kernel (v4).

Reference semantics (T=4, THRESH=1, TAU=1, ALPHA=0.5):
    x: [32, 1024, 512] fp32 -> flat timeline [128 steps, 256, 512]
    per step t:
        mem   = mem + x_t
        sp    = (mem >= 1.0)
        res   = 0.5 * res + sp          # output at step t
        mem   = mem * (1 - sp)

Design (per core, 16384 neurons = 128 partitions x 128 f, 128 steps):
  Two fused custom-DVE ops per step chunk (registered at import):
    LIF_STEP_ANT:  memb_t = memb_{t-1} * (memb_{t-1} < 1) + x_t   (fp32)
    RES_STEP_ANT:  res_t  = res_{t-1} * 0.5 + (memb_t >= 1)       (bf16)
  Self-overlapping ring APs let one instruction stream a whole chunk's
  recurrence at 1 elem/cycle.

  v4 structural changes vs v3:
   - every x chunk gets its own SBUF tile (x fits whole: 8.4MB); all
     input DMAs are issued up-front from the (otherwise idle) PE
     engine's queue, so no input transfer ever waits on compute and
     output-DMA triggers (which wait on RES sems) cannot block input
     issues (head-of-line on the Sync queue was the v3 bottleneck:
     DMA dipped to 33GB/s mid-kernel; burst rate is ~430GB/s).
   - small leading chunks so the DVE chain starts right after the
     first packets land.
   - output DMAs stay on Sync, split finer near the end to shorten
     the drain tail.

Sharding: neuron n_core = p*128 + f; core c owns neurons
[c*16384, (c+1)*16384) -- data-parallel, no cross-core comms.
"""

import numpy as np

N_STEPS = 128
N_NEURONS = 131072
N_CORES = 8
N_PER_CORE = N_NEURONS // N_CORES   # 16384
P = 128                             # SBUF partitions
F = N_PER_CORE // P                 # 128 neurons per partition

_CACHE = {}

# input chunks (steps): small head for a fast DVE start
IN_CHUNKS = [2, 2, 4, 8, 16, 32, 32, 32]
assert sum(IN_CHUNKS) == N_STEPS


def _register_ops():
    """Register the two fused custom DVE ops (idempotent)."""
    import concourse.dve_ops as dve_ops
    from concourse.dve_spec import Spec, Src0, Src1, lower
    from concourse.dve_uop import DveOpSpec

    def reg(name, spec, **kw):
        for o in dve_ops.OPS:
            if o.name == name:
                return o
        row = max(dve_ops._SUB_OPCODE_FOR_NAME.values()) + 1
        assert row < 0x20
        shas = {}
        for ver in ("v3", "v4"):
            d = DveOpSpec(name=name, opcode=row, uops=lower(spec, ver=ver),
                          rd1_en=True, **kw)
            shas[ver] = d.sha(ver)
        op = dve_ops.DveOp(name, spec, subdim=False, uops_sha=shas)
        dve_ops.OPS.append(op)
        dve_ops.CUSTOM_DVE_SPECS[name] = spec
        dve_ops._SUB_OPCODE_FOR_NAME[name] = row
        return op

    def C(leaf):
        return leaf

    from concourse.dve_spec import C0, C1

    lif = reg(
        "LIF_STEP_ANT",
        Spec(
            body=Src0 * (Src0 < C0) + Src1,
            reference=lambda in0, in1, s0, s1, imm2: (
                in0 * (in0 < s0) + in1
            ).astype(np.float32),
        ),
    )
    res = reg(
        "RES_STEP_ANT",
        Spec(
            body=Src0 * C0 + (Src1 >= C1),
            reference=lambda in0, in1, s0, s1, imm2: (
                in0 * s0 + (in1 >= s1)
            ).astype(np.float32),
        ),
    )
    return lif, res


def _build_program():
    import concourse.bacc as bacc
    import concourse.mybir as mybir
    from concourse.tile import TileContext

    f32 = mybir.dt.float32
    bf16 = mybir.dt.bfloat16
    lif, res = _register_ops()

    nc = bacc.Bacc()
    x_d = nc.dram_tensor("x", [P, N_STEPS * F], f32, kind="ExternalInput")
    o_d = nc.dram_tensor("o", [P, N_STEPS * F], bf16, kind="ExternalOutput")

    with TileContext(nc) as tc:
        with (
            tc.tile_pool(name="xin", bufs=1) as xpool,
            tc.tile_pool(name="single", bufs=1) as spool,
        ):
            M = spool.tile([P, N_STEPS + 1, F], f32)   # membrane ring
            R = spool.tile([P, N_STEPS + 1, F], bf16)  # residue ring
            nc.vector.memset(M[:, 0, :], 0.0)
            nc.vector.memset(R[:, 0, :], 0.0)

            # All input DMAs up-front on the PE queue; per-chunk tiles, so
            # nothing ever blocks an input transfer.
            xts = []
            t0 = 0
            for w in IN_CHUNKS:
                xt = xpool.tile([P, w, F], f32)
                nc.gpsimd.dma_start(
                    out=xt[:], in_=x_d[:, t0 * F:(t0 + w) * F]
                )
                xts.append((t0, w, xt))
                t0 += w

            for ci, (t0, w, xt) in enumerate(xts):
                nc.vector._custom_dve(
                    lif, out=M[:, t0 + 1:t0 + 1 + w, :],
                    in0=M[:, t0:t0 + w, :], in1=xt[:], s0=1.0,
                )
                # split the last chunk's residues so the final out-DMA
                # covers only 8 steps
                last = ci == len(xts) - 1
                parts = [(t0, w)] if not last else [
                    (t0, w - 16), (t0 + w - 16, 8), (t0 + w - 8, 8)
                ]
                for r0, rw in parts:
                    nc.vector._custom_dve(
                        res, out=R[:, r0 + 1:r0 + 1 + rw, :],
                        in0=R[:, r0:r0 + rw, :],
                        in1=M[:, r0 + 1:r0 + 1 + rw, :], s0=0.5, s1=1.0,
                    )
                    nc.sync.dma_start(
                        out=o_d[:, r0 * F:(r0 + rw) * F],
                        in_=R[:, r0 + 1:r0 + 1 + rw, :],
                    )
    nc.finalize()
    return nc


def _get_program():
    if "nc" not in _CACHE:
        _CACHE["nc"] = _build_program()
    return _CACHE["nc"]


def _shard_inputs(x: np.ndarray) -> list[np.ndarray]:
    """[32,1024,512] -> per-core [P, N_STEPS*F] partition-major arrays."""
    xf = np.ascontiguousarray(x, dtype=np.float32).reshape(N_STEPS, N_NEURONS)
    shards = []
    for c in range(N_CORES):
        s = xf[:, c * N_PER_CORE:(c + 1) * N_PER_CORE]   # [T, 16384]
        s = s.reshape(N_STEPS, P, F).transpose(1, 0, 2).reshape(
            P, N_STEPS * F
        )
        shards.append(np.ascontiguousarray(s))
    return shards


def _unshard_outputs(outs: list[np.ndarray]) -> np.ndarray:
    """Per-core o [P, T*F] bf16 (t-major) -> [32,1024,512] f32."""
    full = np.empty((N_STEPS, N_NEURONS), dtype=np.float32)
    for c, o in enumerate(outs):
        s = np.asarray(o).astype(np.float32).reshape(P, N_STEPS, F)
        full[:, c * N_PER_CORE:(c + 1) * N_PER_CORE] = (
            s.transpose(1, 0, 2).reshape(N_STEPS, N_PER_CORE)
        )
    return full.reshape(32, 1024, 512)


def kernel(x: np.ndarray) -> np.ndarray:
    from concourse.bass_utils import run_bass_kernel_spmd

    steps, tb, d = x.shape
    assert (steps, tb, d) == (32, 1024, 512), x.shape

    in_maps = [{"x": s} for s in _shard_inputs(x)]
    nc = _get_program()
    res = run_bass_kernel_spmd(nc, in_maps, list(range(N_CORES)))
    return _unshard_outputs(
        [res.results[c]["o"] for c in range(N_CORES)]
    )


# revision 4
# speedup vs baseline: 1.1066x; 1.1066x over previous
"""LIF-with-residue Trainium2 kernel (v4).

Reference semantics (T=4, THRESH=1, TAU=1, ALPHA=0.5):
    x: [32, 1024, 512] fp32 -> flat timeline [128 steps, 256, 512]
    per step t:
        mem   = mem + x_t
        sp    = (mem >= 1.0)
        res   = 0.5 * res + sp          # output at step t
        mem   = mem * (1 - sp)

Design (per core, 16384 neurons = 128 partitions x 128 f, 128 steps):
  Two fused custom-DVE ops per step chunk (registered at import):
    LIF_STEP_ANT:  memb_t = memb_{t-1} * (memb_{t-1} < 1) + x_t   (fp32)
    RES_STEP_ANT:  res_t  = res_{t-1} * 0.5 + (memb_t >= 1)       (bf16)
  Self-overlapping ring APs let one instruction stream a whole chunk's
  recurrence at 1 elem/cycle.

  v4 structural changes vs v3:
   - every x chunk gets its own SBUF tile (x fits whole: 8.4MB); all
     input DMAs are issued up-front from the (otherwise idle) PE
     engine's queue, so no input transfer ever waits on compute and
     output-DMA triggers (which wait on RES sems) cannot block input
     issues (head-of-line on the Sync queue was the v3 bottleneck:
     DMA dipped to 33GB/s mid-kernel; burst rate is ~430GB/s).
   - small leading chunks so the DVE chain starts right after the
     first packets land.
   - output DMAs stay on Sync, split finer near the end to shorten
     the drain tail.

Sharding: neuron n_core = p*128 + f; core c owns neurons
[c*16384, (c+1)*16384) -- data-parallel, no cross-core comms.
"""

import numpy as np

N_STEPS = 128
N_NEURONS = 131072
N_CORES = 8
N_PER_CORE = N_NEURONS // N_CORES   # 16384
P = 128                             # SBUF partitions
F = N_PER_CORE // P                 # 128 neurons per partition

_CACHE = {}

# input chunks (steps): small head for a fast DVE start
IN_CHUNKS = [2, 2, 4, 8, 16, 32, 32, 32]
assert sum(IN_CHUNKS) == N_STEPS


def _register_ops():
    """Register the two fused custom DVE ops (idempotent)."""
    import concourse.dve_ops as dve_ops
    from concourse.dve_spec import Spec, Src0, Src1, lower
    from concourse.dve_uop import DveOpSpec

    def reg(name, spec, **kw):
        for o in dve_ops.OPS:
            if o.name == name:
                return o
        row = max(dve_ops._SUB_OPCODE_FOR_NAME.values()) + 1
        assert row < 0x20
        shas = {}
        for ver in ("v3", "v4"):
            d = DveOpSpec(name=name, opcode=row, uops=lower(spec, ver=ver),
                          rd1_en=True, **kw)
            shas[ver] = d.sha(ver)
        op = dve_ops.DveOp(name, spec, subdim=False, uops_sha=shas)
        dve_ops.OPS.append(op)
        dve_ops.CUSTOM_DVE_SPECS[name] = spec
        dve_ops._SUB_OPCODE_FOR_NAME[name] = row
        return op

    def C(leaf):
        return leaf

    from concourse.dve_spec import C0, C1

    lif = reg(
        "LIF_STEP_ANT",
        Spec(
            body=Src0 * (Src0 < C0) + Src1,
            reference=lambda in0, in1, s0, s1, imm2: (
                in0 * (in0 < s0) + in1
            ).astype(np.float32),
        ),
    )
    res = reg(
        "RES_STEP_ANT",
        Spec(
            body=Src0 * C0 + (Src1 >= C1),
            reference=lambda in0, in1, s0, s1, imm2: (
                in0 * s0 + (in1 >= s1)
            ).astype(np.float32),
        ),
    )
    return lif, res


def _build_program():
    import concourse.bacc as bacc
    import concourse.mybir as mybir
    from concourse.tile import TileContext

    f32 = mybir.dt.float32
    bf16 = mybir.dt.bfloat16
    lif, res = _register_ops()

    nc = bacc.Bacc()
    x_d = nc.dram_tensor("x", [P, N_STEPS * F], f32, kind="ExternalInput")
    o_d = nc.dram_tensor("o", [P, N_STEPS * F], bf16, kind="ExternalOutput")

    with TileContext(nc) as tc:
        with (
            tc.tile_pool(name="xin", bufs=1) as xpool,
            tc.tile_pool(name="single", bufs=1) as spool,
        ):
            M = spool.tile([P, N_STEPS + 1, F], f32)   # membrane ring
            R = spool.tile([P, N_STEPS + 1, F], bf16)  # residue ring
            nc.vector.memset(M[:, 0, :], 0.0)
            nc.vector.memset(R[:, 0, :], 0.0)

            # All input DMAs up-front on the PE queue; per-chunk tiles, so
            # nothing ever blocks an input transfer.
            xts = []
            t0 = 0
            for w in IN_CHUNKS:
                xt = xpool.tile([P, w, F], f32)
                nc.scalar.dma_start(
                    out=xt[:], in_=x_d[:, t0 * F:(t0 + w) * F]
                )
                xts.append((t0, w, xt))
                t0 += w

            for ci, (t0, w, xt) in enumerate(xts):
                nc.vector._custom_dve(
                    lif, out=M[:, t0 + 1:t0 + 1 + w, :],
                    in0=M[:, t0:t0 + w, :], in1=xt[:], s0=1.0,
                )
                # split the last chunk's residues so the final out-DMA
                # covers only 8 steps
                last = ci == len(xts) - 1
                parts = [(t0, w)] if not last else [
                    (t0, w - 16), (t0 + w - 16, 8), (t0 + w - 8, 8)
                ]
                for r0, rw in parts:
                    nc.vector._custom_dve(
                        res, out=R[:, r0 + 1:r0 + 1 + rw, :],
                        in0=R[:, r0:r0 + rw, :],
                        in1=M[:, r0 + 1:r0 + 1 + rw, :], s0=0.5, s1=1.0,
                    )
                    nc.sync.dma_start(
                        out=o_d[:, r0 * F:(r0 + rw) * F],
                        in_=R[:, r0 + 1:r0 + 1 + rw, :],
                    )
    nc.finalize()
    return nc


def _get_program():
    if "nc" not in _CACHE:
        _CACHE["nc"] = _build_program()
    return _CACHE["nc"]


def _shard_inputs(x: np.ndarray) -> list[np.ndarray]:
    """[32,1024,512] -> per-core [P, N_STEPS*F] partition-major arrays."""
    xf = np.ascontiguousarray(x, dtype=np.float32).reshape(N_STEPS, N_NEURONS)
    shards = []
    for c in range(N_CORES):
        s = xf[:, c * N_PER_CORE:(c + 1) * N_PER_CORE]   # [T, 16384]
        s = s.reshape(N_STEPS, P, F).transpose(1, 0, 2).reshape(
            P, N_STEPS * F
        )
        shards.append(np.ascontiguousarray(s))
    return shards


def _unshard_outputs(outs: list[np.ndarray]) -> np.ndarray:
    """Per-core o [P, T*F] bf16 (t-major) -> [32,1024,512] f32."""
    full = np.empty((N_STEPS, N_NEURONS), dtype=np.float32)
    for c, o in enumerate(outs):
        s = np.asarray(o).astype(np.float32).reshape(P, N_STEPS, F)
        full[:, c * N_PER_CORE:(c + 1) * N_PER_CORE] = (
            s.transpose(1, 0, 2).reshape(N_STEPS, N_PER_CORE)
        )
    return full.reshape(32, 1024, 512)


def kernel(x: np.ndarray) -> np.ndarray:
    from concourse.bass_utils import run_bass_kernel_spmd

    steps, tb, d = x.shape
    assert (steps, tb, d) == (32, 1024, 512), x.shape

    in_maps = [{"x": s} for s in _shard_inputs(x)]
    nc = _get_program()
    res = run_bass_kernel_spmd(nc, in_maps, list(range(N_CORES)))
    return _unshard_outputs(
        [res.results[c]["o"] for c in range(N_CORES)]
    )


# revision 5
# speedup vs baseline: 1.5291x; 1.3819x over previous
"""LIF-with-residue Trainium2 kernel (v4).

Reference semantics (T=4, THRESH=1, TAU=1, ALPHA=0.5):
    x: [32, 1024, 512] fp32 -> flat timeline [128 steps, 256, 512]
    per step t:
        mem   = mem + x_t
        sp    = (mem >= 1.0)
        res   = 0.5 * res + sp          # output at step t
        mem   = mem * (1 - sp)

Design (per core, 16384 neurons = 128 partitions x 128 f, 128 steps):
  Two fused custom-DVE ops per step chunk (registered at import):
    LIF_STEP_ANT:  memb_t = memb_{t-1} * (memb_{t-1} < 1) + x_t   (fp32)
    RES_STEP_ANT:  res_t  = res_{t-1} * 0.5 + (memb_t >= 1)       (bf16)
  Self-overlapping ring APs let one instruction stream a whole chunk's
  recurrence at 1 elem/cycle.

  v4 structural changes vs v3:
   - every x chunk gets its own SBUF tile (x fits whole: 8.4MB); all
     input DMAs are issued up-front from the (otherwise idle) PE
     engine's queue, so no input transfer ever waits on compute and
     output-DMA triggers (which wait on RES sems) cannot block input
     issues (head-of-line on the Sync queue was the v3 bottleneck:
     DMA dipped to 33GB/s mid-kernel; burst rate is ~430GB/s).
   - small leading chunks so the DVE chain starts right after the
     first packets land.
   - output DMAs stay on Sync, split finer near the end to shorten
     the drain tail.

Sharding: neuron n_core = p*128 + f; core c owns neurons
[c*16384, (c+1)*16384) -- data-parallel, no cross-core comms.
"""

import numpy as np

N_STEPS = 128
N_NEURONS = 131072
N_CORES = 8
N_PER_CORE = N_NEURONS // N_CORES   # 16384
P = 128                             # SBUF partitions
F = N_PER_CORE // P                 # 128 neurons per partition

_CACHE = {}

# input chunks (steps): small head for a fast DVE start
IN_CHUNKS = [2, 2, 4, 8, 16, 32, 32, 32]
assert sum(IN_CHUNKS) == N_STEPS


def _register_ops():
    """Register the two fused custom DVE ops (idempotent)."""
    import concourse.dve_ops as dve_ops
    from concourse.dve_spec import Spec, Src0, Src1, lower
    from concourse.dve_uop import DveOpSpec

    def reg(name, spec, **kw):
        for o in dve_ops.OPS:
            if o.name == name:
                return o
        row = max(dve_ops._SUB_OPCODE_FOR_NAME.values()) + 1
        assert row < 0x20
        shas = {}
        for ver in ("v3", "v4"):
            d = DveOpSpec(name=name, opcode=row, uops=lower(spec, ver=ver),
                          rd1_en=True, **kw)
            shas[ver] = d.sha(ver)
        op = dve_ops.DveOp(name, spec, subdim=False, uops_sha=shas)
        dve_ops.OPS.append(op)
        dve_ops.CUSTOM_DVE_SPECS[name] = spec
        dve_ops._SUB_OPCODE_FOR_NAME[name] = row
        return op

    def C(leaf):
        return leaf

    from concourse.dve_spec import C0, C1

    lif = reg(
        "LIF_STEP_ANT",
        Spec(
            body=Src0 * (Src0 < C0) + Src1,
            reference=lambda in0, in1, s0, s1, imm2: (
                in0 * (in0 < s0) + in1
            ).astype(np.float32),
        ),
    )
    res = reg(
        "RES_STEP_ANT",
        Spec(
            body=Src0 * C0 + (Src1 >= C1),
            reference=lambda in0, in1, s0, s1, imm2: (
                in0 * s0 + (in1 >= s1)
            ).astype(np.float32),
        ),
    )
    return lif, res


def _build_program():
    import concourse.bacc as bacc
    import concourse.mybir as mybir
    from concourse.tile import TileContext

    f32 = mybir.dt.float32
    bf16 = mybir.dt.bfloat16
    lif, res = _register_ops()

    nc = bacc.Bacc()
    x_d = nc.dram_tensor("x", [P, N_STEPS * F], f32, kind="ExternalInput")
    o_d = nc.dram_tensor("o", [P, N_STEPS * F], bf16, kind="ExternalOutput")

    with TileContext(nc) as tc:
        with (
            tc.tile_pool(name="xin", bufs=1) as xpool,
            tc.tile_pool(name="single", bufs=1) as spool,
        ):
            M = spool.tile([P, N_STEPS + 1, F], f32)   # membrane ring
            R = spool.tile([P, N_STEPS + 1, F], bf16)  # residue ring
            nc.vector.memset(M[:, 0, :], 0.0)
            nc.vector.memset(R[:, 0, :], 0.0)

            # All input DMAs up-front on the PE queue; per-chunk tiles, so
            # nothing ever blocks an input transfer.
            xts = []
            t0 = 0
            for ci, w in enumerate(IN_CHUNKS):
                xt = xpool.tile([P, w, F], f32, name=f"xin{ci}", tag=f"xin{ci}")
                nc.scalar.dma_start(
                    out=xt[:], in_=x_d[:, t0 * F:(t0 + w) * F]
                )
                xts.append((t0, w, xt))
                t0 += w

            for ci, (t0, w, xt) in enumerate(xts):
                nc.vector._custom_dve(
                    lif, out=M[:, t0 + 1:t0 + 1 + w, :],
                    in0=M[:, t0:t0 + w, :], in1=xt[:], s0=1.0,
                )
                # split the last chunk's residues so the final out-DMA
                # covers only 8 steps
                last = ci == len(xts) - 1
                parts = [(t0, w)] if not last else [
                    (t0, w - 16), (t0 + w - 16, 8), (t0 + w - 8, 8)
                ]
                for r0, rw in parts:
                    nc.vector._custom_dve(
                        res, out=R[:, r0 + 1:r0 + 1 + rw, :],
                        in0=R[:, r0:r0 + rw, :],
                        in1=M[:, r0 + 1:r0 + 1 + rw, :], s0=0.5, s1=1.0,
                    )
                    nc.sync.dma_start(
                        out=o_d[:, r0 * F:(r0 + rw) * F],
                        in_=R[:, r0 + 1:r0 + 1 + rw, :],
                    )
    nc.finalize()
    return nc


def _get_program():
    if "nc" not in _CACHE:
        _CACHE["nc"] = _build_program()
    return _CACHE["nc"]


def _shard_inputs(x: np.ndarray) -> list[np.ndarray]:
    """[32,1024,512] -> per-core [P, N_STEPS*F] partition-major arrays."""
    xf = np.ascontiguousarray(x, dtype=np.float32).reshape(N_STEPS, N_NEURONS)
    shards = []
    for c in range(N_CORES):
        s = xf[:, c * N_PER_CORE:(c + 1) * N_PER_CORE]   # [T, 16384]
        s = s.reshape(N_STEPS, P, F).transpose(1, 0, 2).reshape(
            P, N_STEPS * F
        )
        shards.append(np.ascontiguousarray(s))
    return shards


def _unshard_outputs(outs: list[np.ndarray]) -> np.ndarray:
    """Per-core o [P, T*F] bf16 (t-major) -> [32,1024,512] f32."""
    full = np.empty((N_STEPS, N_NEURONS), dtype=np.float32)
    for c, o in enumerate(outs):
        s = np.asarray(o).astype(np.float32).reshape(P, N_STEPS, F)
        full[:, c * N_PER_CORE:(c + 1) * N_PER_CORE] = (
            s.transpose(1, 0, 2).reshape(N_STEPS, N_PER_CORE)
        )
    return full.reshape(32, 1024, 512)


def kernel(x: np.ndarray) -> np.ndarray:
    from concourse.bass_utils import run_bass_kernel_spmd

    steps, tb, d = x.shape
    assert (steps, tb, d) == (32, 1024, 512), x.shape

    in_maps = [{"x": s} for s in _shard_inputs(x)]
    nc = _get_program()
    res = run_bass_kernel_spmd(nc, in_maps, list(range(N_CORES)))
    return _unshard_outputs(
        [res.results[c]["o"] for c in range(N_CORES)]
    )
